# revision 4
# baseline (speedup 1.0000x reference)
"""Trainium2 Bass kernel for nn_Attention (GQA with group-summed query heads).

Algorithm notes (validated against reference in numpy):
- The reference einsum 'bghnd,bhsd->bhns' SUMS over the query-group axis, so the
  16 query heads collapse into 4 effective heads: wq columns can be pre-summed
  per kv-head (RoPE is linear per-position, both /sqrt(64) scalings folded in).
- This makes the problem plain 4-head attention: 2 batches x 4 kv-heads = 8
  independent (b,h) attention instances -> one per NeuronCore.
- Head dims are pair-permuted [t1(even), t2(odd)] so RoPE becomes two wide
  elementwise multiply-adds against [cos;cos;cos;cos] / [sin;sin;sin;sin]
  tiles, using two projection matmuls P1 = W1.T@xT, P2 = W2.T@xT where W2 has
  the pair-swapped, sign-folded columns.
- Scores are computed TRANSPOSED (scoresT[key, query]) so exp(scoresT) is
  directly usable as the AV matmul's stationary-side rhs with V as lhsT; an
  all-ones column appended to V yields the softmax denominators for free.
- No max-subtraction needed: scores = q_eff . k / 64 with |scores| <~ 1.5.
- Final: 8-core AllToAll redistributes per-head outputs into 512-row output
  slices; each core normalizes and applies the row-parallel wo matmul.
"""

import sys
import os

sys.path.insert(0, "/opt/trn_rl_repo")

import numpy as np
import ml_dtypes

B, S, D = 2, 4096, 1024
QH, KVH, HD = 16, 4, 64
KV_DIM = KVH * HD  # 256
NCORES = 8
SB = 512           # s-block / q-block width
NSB = S // SB      # 8
GRP = 3            # key-chunks (128 keys each) per exp group -> [128, 1536] psum

_CACHE = {}


def _build_nc():
    import concourse.bacc as bacc
    import concourse.tile as tile
    from concourse import mybir

    f32 = mybir.dt.float32
    bf = mybir.dt.bfloat16
    EXP = mybir.ActivationFunctionType.Exp

    nc = bacc.Bacc("TRN2", target_bir_lowering=False, debug=False,
                   num_devices=NCORES)

    xT_d = nc.dram_tensor("xT", [D, S], f32, kind="ExternalInput")
    w1_d = nc.dram_tensor("w1", [D, 128], f32, kind="ExternalInput")
    w2_d = nc.dram_tensor("w2", [D, 128], f32, kind="ExternalInput")
    wv_d = nc.dram_tensor("wv", [D, HD], f32, kind="ExternalInput")
    ab_d = nc.dram_tensor("ab", [2, 128, S], bf, kind="ExternalInput")
    mk_d = nc.dram_tensor("masks", [4, 128, SB], bf, kind="ExternalInput")
    wo_d = nc.dram_tensor("wo", [KV_DIM, D], f32, kind="ExternalInput")
    out_d = nc.dram_tensor("out", [B, SB, D], f32, kind="ExternalOutput")

    idn_d = nc.inline_tensor(np.eye(HD, dtype=np.float32), "idn")
    ones_d = nc.inline_tensor(np.ones((1, HD), np.float32), "ones64")

    with tile.TileContext(nc) as tc:
        with (
            tc.tile_pool(name="persist", bufs=1) as pp,
            tc.tile_pool(name="work", bufs=2) as wp,
            tc.tile_pool(name="expp", bufs=2) as ep,
            tc.tile_pool(name="ps_sc", bufs=2, space="PSUM") as ps_sc,
            tc.tile_pool(name="ps_aux", bufs=1, space="PSUM") as ps_aux,
            tc.tile_pool(name="dram", bufs=1, space="DRAM") as dp,
        ):
            # ---- persistent SBUF tensors ----
            xT = pp.tile([128, 8 * S], bf, tag="xT")          # 64KB/part
            w1 = pp.tile([128, 8 * 128], bf, tag="w1")
            w2 = pp.tile([128, 8 * 128], bf, tag="w2")
            wv = pp.tile([128, 8 * HD], bf, tag="wv")
            At = pp.tile([128, S], bf, tag="At")
            Bt = pp.tile([128, S], bf, tag="Bt")
            mk = pp.tile([128, 4 * SB], bf, tag="mk")
            wo = pp.tile([HD, KVH * D], bf, tag="wo")         # head h at cols D*h
            rot = pp.tile([128, S], bf, tag="rot")            # rows 0:64 q~, 64:128 k~
            rotk = pp.tile([HD, S], bf, tag="rotk")           # k~ at base partition 0
            vaug = pp.tile([128, 32 * (HD + 1)], bf, tag="vaug")
            outT = pp.tile([HD + 1, S], f32, tag="outT")
            idn = pp.tile([HD, HD], f32, tag="idn")
            ones = pp.tile([1, HD], f32, tag="ones")
            sums = pp.tile([1, NCORES * SB], f32, tag="sums")

            # ---- input loads (gpsimd = SWDGE casts f32->bf16 in flight) ----
            for d8 in range(8):
                nc.gpsimd.dma_start(xT[:, S * d8: S * (d8 + 1)],
                                    xT_d[128 * d8: 128 * (d8 + 1), :])
            for c in range(8):
                nc.gpsimd.dma_start(w1[:, 128 * c: 128 * (c + 1)],
                                    w1_d[128 * c: 128 * (c + 1), :])
                nc.gpsimd.dma_start(w2[:, 128 * c: 128 * (c + 1)],
                                    w2_d[128 * c: 128 * (c + 1), :])
                nc.gpsimd.dma_start(wv[:, HD * c: HD * (c + 1)],
                                    wv_d[128 * c: 128 * (c + 1), :])
            for h in range(KVH):
                nc.gpsimd.dma_start(wo[:, D * h: D * (h + 1)],
                                    wo_d[HD * h: HD * (h + 1), :])
            nc.sync.dma_start(At[:, :], ab_d[0, :, :])
            nc.sync.dma_start(Bt[:, :], ab_d[1, :, :])
            for r in range(4):
                nc.sync.dma_start(mk[:, SB * r: SB * (r + 1)], mk_d[r, :, :])
            nc.sync.dma_start(idn[:, :], idn_d[:, :])
            nc.sync.dma_start(ones[:, :], ones_d[:, :])
            for c in range(32):
                nc.vector.memset(vaug[:, (HD + 1) * c + HD: (HD + 1) * (c + 1)], 1.0)

            # ---- stage P: projections + RoPE + V transpose ----
            for j in range(NSB):
                sc = ps_sc.tile([128, 3 * SB], f32, tag="sc")

                def xs(d8, _j=j):
                    return xT[:, S * d8 + SB * _j: S * d8 + SB * (_j + 1)]

                for d8 in range(8):
                    nc.tensor.matmul(sc[:, 0:512], w1[:, 128 * d8: 128 * (d8 + 1)],
                                     xs(d8), start=(d8 == 0), stop=(d8 == 7))
                for d8 in range(8):
                    nc.tensor.matmul(sc[:, 512:1024], w2[:, 128 * d8: 128 * (d8 + 1)],
                                     xs(d8), start=(d8 == 0), stop=(d8 == 7))
                for d8 in range(8):
                    nc.tensor.matmul(sc[0:HD, 1024:1536], wv[:, HD * d8: HD * (d8 + 1)],
                                     xs(d8), start=(d8 == 0), stop=(d8 == 7))

                u = wp.tile([128, SB], f32, tag="u")
                w_ = wp.tile([128, SB], f32, tag="w_")
                nc.vector.tensor_mul(u[:, :], sc[:, 0:512], At[:, SB * j: SB * (j + 1)])
                nc.vector.tensor_mul(w_[:, :], sc[:, 512:1024], Bt[:, SB * j: SB * (j + 1)])
                nc.vector.tensor_add(rot[:, SB * j: SB * (j + 1)], u[:, :], w_[:, :])
                # k~ copy down to base partition 0 (DMA moves across partitions)
                nc.sync.dma_start(rotk[:, SB * j: SB * (j + 1)],
                                  rot[64:128, SB * j: SB * (j + 1)])

                vts = wp.tile([HD, SB], f32, tag="vts")
                nc.any.tensor_copy(vts[:, :], sc[0:HD, 1024:1536])
                vtr = ps_aux.tile([128, 512], f32, tag="vtr")
                for t in range(4):
                    nc.tensor.transpose(vtr[:, HD * t: HD * (t + 1)],
                                        vts[:, 128 * t: 128 * (t + 1)], idn[:, :])
                for t in range(4):
                    cch = 4 * j + t
                    nc.any.tensor_copy(vaug[:, (HD + 1) * cch: (HD + 1) * cch + HD],
                                       vtr[:, HD * t: HD * (t + 1)])

            # ---- stage A: attention (scoresT -> exp -> masked -> AV) ----
            for qb in range(NSB):
                po = ps_aux.tile([HD + 1, 512], f32, tag="po")
                nk = 4 * (qb + 1)
                for g0 in range(0, nk, GRP):
                    cnt = min(GRP, nk - g0)
                    sc = ps_sc.tile([128, 3 * SB], f32, tag="sc")
                    for r in range(cnt):
                        kb = g0 + r
                        nc.tensor.matmul(sc[:, 512 * r: 512 * (r + 1)],
                                         rotk[:, 128 * kb: 128 * (kb + 1)],
                                         rot[0:HD, SB * qb: SB * (qb + 1)],
                                         start=True, stop=True)
                    pe = ep.tile([128, 3 * SB], bf, tag="pe")
                    nc.scalar.activation(pe[:, 0: 512 * cnt], sc[:, 0: 512 * cnt], EXP)
                    for r in range(cnt):
                        di = (g0 + r) - (nk - 4)
                        if di >= 0:
                            nc.vector.tensor_mul(pe[:, 512 * r: 512 * (r + 1)],
                                                 pe[:, 512 * r: 512 * (r + 1)],
                                                 mk[:, SB * di: SB * (di + 1)])
                    for r in range(cnt):
                        kb = g0 + r
                        nc.tensor.matmul(po[:, :],
                                         vaug[:, (HD + 1) * kb: (HD + 1) * kb + HD + 1],
                                         pe[:, 512 * r: 512 * (r + 1)],
                                         start=(kb == 0), stop=(kb == nk - 1))
                nc.any.tensor_copy(outT[:, SB * qb: SB * (qb + 1)], po[:, :])

            # ---- stage C: AllToAll (512-col output slices across all 8 cores) ----
            bin_ = dp.tile([NCORES, HD + 1, 512], f32, tag="bin")
            bout = dp.tile([NCORES, HD + 1, 512], f32, tag="bout")
            for c in range(NCORES):
                nc.sync.dma_start(bin_[c, :, :], outT[:, 512 * c: 512 * (c + 1)])
            from concourse import mybir as _mb
            nc.gpsimd.collective_compute(
                "AllToAll", _mb.AluOpType.bypass,
                replica_groups=[list(range(NCORES))],
                ins=[bin_.opt()], outs=[bout.opt()],
            )

            # ---- stage D: normalize + wo matmul + output ----
            gs = []
            for c in range(NCORES):
                g = pp.tile([HD, 512], bf, tag=f"g{c}")
                nc.gpsimd.dma_start(g[:, :], bout[c, 0:HD, :])
                nc.sync.dma_start(sums[:, 512 * c: 512 * (c + 1)], bout[c, HD:HD + 1, :])
                gs.append(g)
            nc.vector.reciprocal(sums[:, :], sums[:, :])
            nts = []
            for c in range(NCORES):
                bc = ps_aux.tile([HD, 512], f32, tag="vtr")
                nc.tensor.matmul(bc[:, :], ones[:, :], sums[:, 512 * c: 512 * (c + 1)],
                                 start=True, stop=True)
                nt = pp.tile([HD, 512], bf, tag=f"nt{c}")
                nc.vector.tensor_mul(nt[:, :], gs[c][:, :], bc[:, :])
                nts.append(nt)
            for b in range(B):
                for t in range(4):
                    for nh in range(2):
                        yp = ps_aux.tile([128, 512], f32, tag="po")
                        for h in range(KVH):
                            nt = nts[KVH * b + h]
                            nc.tensor.matmul(yp[:, :], nt[:, 128 * t: 128 * (t + 1)],
                                             wo[:, D * h + 512 * nh: D * h + 512 * (nh + 1)],
                                             start=(h == 0), stop=(h == KVH - 1))
                        ys = wp.tile([128, 512], f32, tag="ys")
                        nc.any.tensor_copy(ys[:, :], yp[:, :])
                        nc.sync.dma_start(
                            out_d[b, 128 * t: 128 * (t + 1), 512 * nh: 512 * (nh + 1)],
                            ys[:, :])

    nc.compile()
    return nc


def _get_nc():
    if "nc" not in _CACHE:
        _CACHE["nc"] = _build_nc()
    return _CACHE["nc"]


def _prep_in_maps(x, wq, wk, wv, wo, freq_cos, freq_sin):
    x = np.asarray(x, np.float32)
    wq = np.asarray(wq, np.float32)
    wk = np.asarray(wk, np.float32)
    wv = np.asarray(wv, np.float32)
    wo = np.asarray(wo, np.float32)
    cos = np.asarray(freq_cos, np.float32)
    sin = np.asarray(freq_sin, np.float32)

    # group-sum wq per kv head (einsum sums over group axis); fold both /8 scales
    wqr = wq.reshape(D, QH, HD)
    wq_eff = np.stack([wqr[:, h::KVH].sum(axis=1) for h in range(KVH)], axis=1) / 64.0
    wkr = wk.reshape(D, KVH, HD)
    W1 = np.empty((KVH, D, 128), np.float32)
    W2 = np.empty((KVH, D, 128), np.float32)
    for h in range(KVH):
        q1, q2 = wq_eff[:, h, 0::2], wq_eff[:, h, 1::2]
        k1, k2 = wkr[:, h, 0::2], wkr[:, h, 1::2]
        W1[h] = np.concatenate([q1, q2, k1, k2], axis=1)
        W2[h] = np.concatenate([-q2, q1, -k2, k1], axis=1)
    Wv = np.ascontiguousarray(wv.reshape(D, KVH, HD).transpose(1, 0, 2))

    A = np.tile(cos.T, (4, 1)).astype(ml_dtypes.bfloat16)   # [128, S]
    Bm = np.tile(sin.T, (4, 1)).astype(ml_dtypes.bfloat16)
    ab = np.ascontiguousarray(np.stack([A, Bm]))

    qi = np.arange(SB)[None, :]
    ki = np.arange(128)[:, None]
    masks = np.ascontiguousarray(
        np.stack([(qi >= ki + 128 * r) for r in range(4)]).astype(ml_dtypes.bfloat16))

    xTb = [np.ascontiguousarray(x[b].T) for b in range(B)]

    in_maps = []
    for c in range(NCORES):
        b, h = c // KVH, c % KVH
        in_maps.append({
            "xT": xTb[b],
            "w1": np.ascontiguousarray(W1[h]),
            "w2": np.ascontiguousarray(W2[h]),
            "wv": Wv[h],
            "ab": ab,
            "masks": masks,
            "wo": wo,
        })
    return in_maps


def _assemble(results):
    full = np.empty((B, S, D), np.float32)
    for c in range(NCORES):
        y = results[c]["out"]  # [B, 512, D]
        for b in range(B):
            full[b, SB * c: SB * (c + 1), :] = y[b]
    return full


def _ensure_axon_hooks_stub():
    # slim axon builds lack antenv.axon_hooks; degrade trace=True gracefully
    try:
        import antenv.axon_hooks  # noqa: F401
    except Exception:
        import types
        m = types.ModuleType("antenv.axon_hooks")
        m.get_axon_ntff_profile_hook = lambda: None
        sys.modules["antenv.axon_hooks"] = m


def run(in_maps, trace=False):
    from concourse.bass_utils import run_bass_kernel_spmd
    _ensure_axon_hooks_stub()
    nc = _get_nc()
    res = run_bass_kernel_spmd(nc, in_maps, core_ids=list(range(NCORES)),
                               trace=trace)
    return res


def kernel(**inputs):
    in_maps = _prep_in_maps(**inputs)
    res = run(in_maps, trace=False)
    return _assemble(res.results)


if __name__ == "__main__":
    # smoke: build only
    _get_nc()
    print("built ok")
